# revision 33
# baseline (speedup 1.0000x reference)
"""CP-factorized multi-head attention kernel for Trainium2 (8 NeuronCores).

Sharding: data-parallel over batch B=8, one batch element per core.
Per core, per head: S = U Tk^T (rank-64 logits, |S| <= 0.35, so exp needs
no max-subtraction), P = exp(S), O = P V / Z with Z from a ones-column
folded into V. All heavy matmuls fp32r (1 col/cycle); exp on the scalar
engine writes fp32r directly.

vs the original structure:
  - inputs DMA'd as fp32r directly (no SWDGE cast pass), xt on the sync
    queue and weights on the gpsimd queue in first-use order
  - pu packed 2 heads per matmul (block-diag M pairs, duplicated-Tq rhs)
  - the Z broadcast matmul is gone: reciprocal on the [1,512] Z row, then
    a partition-broadcast DMA, then one fused (po * rzb) vector op
  - output projection for i-half 0 is interleaved before attention of
    i-half 1, shrinking the tensor-idle tail
"""

import sys

sys.path.insert(0, "/opt/trn_rl_repo")

import os
import numpy as np
from contextlib import ExitStack

import concourse.bass as bass
from concourse import bacc
import concourse.mybir as mybir
import concourse.tile as tile
from concourse.bass_utils import run_bass_kernel_spmd

FP32 = mybir.dt.float32
FP32R = mybir.dt.float32r
EXP = mybir.ActivationFunctionType.Exp
MULT = mybir.AluOpType.mult

B, N, DIM, H, HD, R = 8, 1024, 768, 12, 64, 64
NCORES = 8

LAST_EXEC_NS = None


def _build_nc():
    nc = bacc.Bacc(
        "TRN2", target_bir_lowering=False, debug=False, num_devices=NCORES
    )
    xt_d = nc.dram_tensor("xt", [DIM, N], FP32R, kind="ExternalInput")
    aqk_d = nc.dram_tensor("a_qk", [128, 768], FP32R, kind="ExternalInput")
    av_d = nc.dram_tensor("a_v", [128, 768], FP32R, kind="ExternalInput")
    m2_d = nc.dram_tensor("m2", [128, 768], FP32R, kind="ExternalInput")
    bv_d = nc.dram_tensor("bv", [64, 768], FP32R, kind="ExternalInput")
    pwt_d = nc.dram_tensor("pwt", [DIM, DIM], FP32R, kind="ExternalInput")
    bias_d = nc.dram_tensor("bias", [768], FP32, kind="ExternalInput")
    z_d = nc.dram_tensor("zeros", [128, 1024], FP32, kind="ExternalInput")
    ov_d = nc.dram_tensor("onesv", [128, H, 1], FP32, kind="ExternalInput")
    oc_d = nc.dram_tensor("onescol", [128, 64], FP32, kind="ExternalInput")
    out_d = nc.dram_tensor("out", [N, DIM], FP32, kind="ExternalOutput")

    with tile.TileContext(nc) as tc, ExitStack() as ctx:
        sing = ctx.enter_context(tc.tile_pool(name="sing", bufs=1))
        psum = ctx.enter_context(tc.tile_pool(name="psum", bufs=2, space="PSUM"))
        work = ctx.enter_context(tc.tile_pool(name="work", bufs=3))
        zp = ctx.enter_context(tc.tile_pool(name="zp", bufs=2))
        rzp = ctx.enter_context(tc.tile_pool(name="rzp", bufs=2))
        ocp = ctx.enter_context(tc.tile_pool(name="ocp", bufs=3))
        obuf = ctx.enter_context(tc.tile_pool(name="obuf", bufs=2))

        xt_sb = [sing.tile([128, 1024], FP32R, tag=f"xt{k}", name=f"xt{k}") for k in range(6)]
        aqk_sb = sing.tile([128, 768], FP32R, tag="aqk")
        av_sb = sing.tile([128, 768], FP32R, tag="av")
        m2_sb = sing.tile([128, 768], FP32R, tag="m2")
        bv_sb = sing.tile([64, 768], FP32R, tag="bv")
        pw_sb = [sing.tile([128, 768], FP32R, tag=f"pw{k}", name=f"pw{k}") for k in range(6)]
        b_sb = sing.tile([128, 768], FP32, tag="b")

        tq2_sb = sing.tile([128, 1024], FP32R, tag="tq2")
        tk_sb = sing.tile([128, 1024], FP32R, tag="tk")
        tv_sb = sing.tile([64, 1024], FP32R, tag="tv")
        v_sb = [sing.tile([128, H, 65], FP32R, tag=f"v{t}", name=f"v{t}") for t in range(8)]
        ot_sb = [sing.tile([128, 1024], FP32R, tag=f"ot{k}", name=f"ot{k}") for k in range(6)]
        # manual ring for U tiles: junk rows 64:128 are zeroed once so the
        # padded contraction rows (tk rows 64:128 = 0) never meet inf/nan
        u_sb = [sing.tile([128, 512], FP32R, tag=f"u{i}", name=f"u{i}") for i in range(4)]
        ocol_sb = sing.tile([128, 64], FP32R, tag="ocol")
        rz_sb = [sing.tile([128, 512], FP32R, tag=f"rz{i}", name=f"rz{i}") for i in range(2)]

        # ---- input DMAs spread across engine queues ----
        nc.sync.dma_start(out=aqk_sb, in_=aqk_d[:, :])
        nc.scalar.dma_start(out=av_sb, in_=av_d[:, :])
        qs = [nc.sync, nc.scalar, nc.gpsimd]
        qi = 0
        for lc in range(2):
            for k in range(6):
                qs[qi % 3].dma_start(
                    out=xt_sb[k][:, lc * 512:(lc + 1) * 512],
                    in_=xt_d[k * 128:(k + 1) * 128, lc * 512:(lc + 1) * 512],
                )
                qi += 1
            if lc == 0:
                nc.gpsimd.dma_start(out=bv_sb, in_=bv_d[:, :])
                nc.gpsimd.dma_start(out=m2_sb, in_=m2_d[:, :])
        nc.gpsimd.dma_start(out=tk_sb[64:128, :], in_=z_d[0:64, :])
        for i in range(4):
            nc.gpsimd.dma_start(out=u_sb[i][64:128, :], in_=z_d[64:128, 0:512])
        for t in range(8):
            nc.gpsimd.dma_start(out=v_sb[t][:, :, 64:65], in_=ov_d[:, :, :])
        nc.gpsimd.dma_start(out=ocol_sb, in_=oc_d[:, :])
        for i in range(2):
            nc.gpsimd.dma_start(out=rz_sb[i][1:128, :], in_=z_d[0:127, 0:512])
        for k in range(6):
            nc.gpsimd.dma_start(out=pw_sb[k], in_=pwt_d[k * 128:(k + 1) * 128, :])
        nc.gpsimd.dma_start(
            out=b_sb, in_=bass.AP(tensor=bias_d, offset=0, ap=[[0, 128], [1, 768]])
        )


        # ---- T-phase: Tq^T (duplicated), Tk^T, Tv^T ----
        for lc in range(2):
            sl = slice(lc * 512, (lc + 1) * 512)
            ptqk = psum.tile([128, 512], FP32, tag="sm", name="ptqk")
            ptv = psum.tile([128, 512], FP32, tag="sm", name="ptv")
            for k in range(6):
                nc.tensor.matmul(
                    ptqk, aqk_sb[:, k * 128:(k + 1) * 128], xt_sb[k][:, sl],
                    start=(k == 0), stop=(k == 5),
                )
                nc.tensor.matmul(
                    ptv, av_sb[:, k * 128:(k + 1) * 128], xt_sb[k][:, sl],
                    start=(k == 0), stop=(k == 5),
                )
            nc.vector.tensor_copy(tq2_sb[0:64, sl], ptqk[0:64, :])
            nc.vector.tensor_copy(tq2_sb[64:128, sl], ptqk[0:64, :])
            nc.vector.tensor_copy(tk_sb[0:64, sl], ptqk[64:128, :])
            nc.scalar.activation(out=tv_sb[:, sl], in_=ptv[0:64, :],
                                 func=mybir.ActivationFunctionType.Copy)

        # ---- V-assembly: V[j, (h, c)] with ones column; lt 2..7 are
        # emitted lazily inside the first head's jp loop ----
        def _emit_v(lt):
            jsl = slice(lt * 128, (lt + 1) * 128)
            for c0, csz in ((0, 512), (512, 256)):
                pv = psum.tile([128, 512], FP32, tag="sm", name="pv")
                nc.tensor.matmul(
                    pv[:, 0:csz], tv_sb[:, jsl], bv_sb[:, c0:c0 + csz],
                    start=True, stop=True,
                )
                h0, nh = c0 // 64, csz // 64
                if lt % 2 == 0:
                    nc.vector.tensor_copy(
                        v_sb[lt][:, h0:h0 + nh, 0:64],
                        pv[:, 0:csz].rearrange("p (h d) -> p h d", d=64),
                    )
                else:
                    nc.scalar.activation(
                        out=v_sb[lt][:, h0:h0 + nh, 0:64],
                        in_=pv[:, 0:csz].rearrange("p (h d) -> p h d", d=64),
                        func=mybir.ActivationFunctionType.Copy,
                    )

        for lt in range(2):
            _emit_v(lt)

        def _emit_norm(oc, rz, kk, half, isl):
            pz = psum.tile([128, 512], FP32, tag="sm", name="pz")
            nc.tensor.matmul(pz[0:64, :], ocol_sb, rz, start=True, stop=True)
            nc.vector.tensor_tensor(
                out=ot_sb[kk][half:half + 64, isl],
                in0=oc[half:half + 64, :], in1=pz[0:64, :], op=MULT,
            )

        def _emit_proj(lt):
            ob = obuf.tile([128, 768], FP32, tag="ob")
            for c0, csz in ((0, 512), (512, 256)):
                pout = psum.tile([128, 512], FP32, tag="sm", name="pout")
                for k in range(6):
                    nc.tensor.matmul(
                        pout[:, 0:csz], ot_sb[k][:, lt * 128:(lt + 1) * 128],
                        pw_sb[k][:, c0:c0 + csz], start=(k == 0), stop=(k == 5),
                    )
                nc.vector.tensor_add(
                    ob[:, c0:c0 + csz], pout[:, 0:csz], b_sb[:, c0:c0 + csz]
                )
            nc.sync.dma_start(out=out_d[lt * 128:(lt + 1) * 128, :], in_=ob)

        # ---- attention + interleaved projection ----
        pending = None
        for ic in range(2):
            isl = slice(ic * 512, (ic + 1) * 512)
            for p in range(6):
                pu = psum.tile([128, 512], FP32, tag="sm", name="pu")
                nc.tensor.matmul(
                    pu, m2_sb[:, p * 128:(p + 1) * 128], tq2_sb[:, isl],
                    start=True, stop=True,
                )
                ub = (ic * 6 + p) * 2
                us = [u_sb[ub % 4], u_sb[(ub + 1) % 4]]
                for hh in range(2):
                    nc.vector.tensor_copy(us[hh][0:64, :], pu[64 * hh:64 * hh + 64, :])
                if ic == 1 and 1 <= p <= 4:
                    if pending is not None:
                        _emit_norm(*pending)
                        pending = None
                    _emit_proj(p - 1)
                for hh in range(2):
                    h = 2 * p + hh
                    kk, half = h // 2, (h % 2) * 64
                    if pending is not None:
                        _emit_norm(*pending)
                        pending = None
                    po = psum.tile([128, 512], FP32, tag="po", name="po")
                    for jp in range(4):
                        ps = psum.tile([128, 1024], FP32, tag="big", name="ps")
                        for t in range(2):
                            jt = 2 * jp + t
                            nc.tensor.matmul(
                                ps[:, t * 512:(t + 1) * 512],
                                tk_sb[:, jt * 128:(jt + 1) * 128], us[hh],
                                start=True, stop=True,
                            )
                        if ic == 0 and p == 0 and hh == 0 and jp < 3:
                            _emit_v(2 * jp + 2)
                            _emit_v(2 * jp + 3)
                        pt = work.tile([128, 1024], FP32R, tag="pt")
                        nc.scalar.activation(out=pt, in_=ps, func=EXP,
                                             bias=0.0, scale=1.0)
                        for t in range(2):
                            jt = 2 * jp + t
                            nc.tensor.matmul(
                                po[0:65, :], v_sb[jt][:, h, :],
                                pt[:, t * 512:(t + 1) * 512],
                                start=(jt == 0), stop=(jt == 7),
                            )
                    # copy po out fast (frees the PSUM bank); recip now,
                    # pz broadcast matmul + final mul deferred one head
                    oc = ocp.tile([128, 512], FP32, tag="oc", name="oc")
                    nc.vector.tensor_copy(oc[half:half + 64, :], po[0:64, :])
                    zrow = zp.tile([1, 512], FP32, tag="zrow", name="zrow")
                    nc.vector.tensor_copy(zrow, po[64:65, :])
                    rzt = zp.tile([1, 512], FP32, tag="rzt", name="rzt")
                    nc.vector.reciprocal_approx_fast(out=rzt, in_=zrow)
                    rz = rz_sb[h % 2]
                    nc.vector.tensor_copy(rz[0:1, :], rzt)
                    pending = (oc, rz, kk, half, isl)
        if pending is not None:
            _emit_norm(*pending)
            pending = None
        for lt in range(4, 8):
            _emit_proj(lt)

    nc.finalize()
    return nc


def _prep_shared(inputs):
    def comb(W1, W2):
        return np.ascontiguousarray(
            (np.asarray(W1, np.float32)[:, None, :]
             * np.asarray(W2, np.float32)[None, :, :]).reshape(DIM, R)
        )

    Aq = comb(inputs["W_Q1"], inputs["W_Q2"])
    Ak = comb(inputs["W_K1"], inputs["W_K2"])
    Av = comb(inputs["W_V1"], inputs["W_V2"])
    a_qk = np.concatenate([Aq, Ak], axis=1)  # [768, 128]
    a_qk_r = np.ascontiguousarray(
        a_qk.reshape(6, 128, 128).transpose(1, 0, 2).reshape(128, 768)
    )
    av_pad = np.zeros((DIM, 128), np.float32)
    av_pad[:, 0:R] = Av
    a_v_r = np.ascontiguousarray(
        av_pad.reshape(6, 128, 128).transpose(1, 0, 2).reshape(128, 768)
    )
    W_Q0 = np.asarray(inputs["W_Q0"], np.float32)
    W_K0 = np.asarray(inputs["W_K0"], np.float32)
    W_V0 = np.asarray(inputs["W_V0"], np.float32)
    scale = HD ** -0.5
    m2 = np.zeros((128, 768), np.float32)
    for h in range(H):
        sl = slice(h * HD, (h + 1) * HD)
        M = scale * (W_Q0[sl, :].T @ W_K0[sl, :])
        pp, half = h // 2, (h % 2) * 64
        m2[half:half + 64, pp * 128 + half:pp * 128 + half + 64] = M
    bv = np.ascontiguousarray(W_V0.T)  # [64, 768]
    pwt = np.ascontiguousarray(np.asarray(inputs["proj_w"], np.float32).T)
    bias = np.asarray(inputs["proj_b"], np.float32)
    zeros = np.zeros((128, 1024), np.float32)
    onesv = np.ones((128, H, 1), np.float32)
    onescol = np.zeros((128, 64), np.float32)
    onescol[0, :] = 1.0
    return dict(a_qk=a_qk_r, a_v=a_v_r, m2=m2, bv=bv, pwt=pwt, bias=bias,
                zeros=zeros, onesv=onesv, onescol=onescol)


def kernel(**inputs) -> np.ndarray:
    global LAST_EXEC_NS
    x = np.asarray(inputs["x"], np.float32)
    shared = _prep_shared(inputs)
    in_maps = []
    for b in range(B):
        m = dict(shared)
        m["xt"] = np.ascontiguousarray(x[b].T)
        in_maps.append(m)

    nc = _build_nc()
    trace = os.environ.get("KERNEL_TRACE", "0") == "1"
    res = run_bass_kernel_spmd(nc, in_maps, core_ids=list(range(NCORES)),
                               trace=trace)
    LAST_EXEC_NS = res.exec_time_ns
    out = np.stack([res.results[i]["out"] for i in range(NCORES)], axis=0)
    return out.astype(np.float32)
